# revision 35
# baseline (speedup 1.0000x reference)
"""Multi-head attention (QKV proj + per-head bias + softmax + out proj) on 8 TRN2 NeuronCores.

Sharding: data-parallel over batch B=4 x tensor-parallel over head-groups
(12 heads -> 2 groups of 6). core = b*2 + hg. Each core computes its 6 heads'
full attention for one batch element plus the partial output projection over
its heads' rows of W_proj; the two partials per batch are summed on the host
(the deferred all-reduce), where b_proj is also added.

Device-side layout notes:
- Everything runs transposed (feature dim on partitions): x^T, q^T, k^T feed
  the PE directly; softmax runs on S^T tiles [m(keys) x n(queries)] so exp is
  elementwise and the denominator comes free from an extra ones-column in the
  AV matmul's stationary operand ([v | 1] -> U rows 0..63 = unnormalized out,
  row 64 = sum of exp). Normalization multiplies by 1/denom broadcast across
  partitions via gpsimd.partition_broadcast.
- All matmul operands are fp16: on real TRN2 silicon fp32/f32r moving
  operands stream at ~2.2 cycles/row while 16-bit streams at 1 cycle/row
  (the CoreSim cost model claims f32r is full-rate; hardware disagrees).
  PSUM accumulation stays fp32. exps are computed as exp(s*SCALE - 8) so
  fp16 can't overflow (the shift cancels in the softmax ratio since the
  denominator from the ones column is scaled identically).
- The kernel is ACT(exp)-bound: 192 steps x ~1.1us per [128,1024] ACTIVATE.
  So the QKV prologue is cut into per-chunk pieces emitted just-in-time into
  the attention stream (emission deadlines keyed to the step that consumes
  each piece), and the output projection runs as three c3-major passes
  accumulated in SBUF so only the last pass (6 matmuls + adds + DMA) trails
  the final attention step.
"""

import numpy as np

import concourse.bacc as bacc
import concourse.tile as tile
from concourse.tile import add_dep_helper
import concourse.mybir as mybir
from concourse.bass_utils import run_bass_kernel_spmd

B, N, C, H, HD = 4, 2048, 768, 12, 64
HL = 6                 # heads per core
CL = HL * HD           # 384 local qkv width
SCALE = HD ** -0.5
P = 128
NB = 512               # query-block (n) size
NBS = N // NB          # 4
MC = N // P            # 16 key-chunks (m)
KC = C // P            # 6 contraction chunks of C
PAIRS = HL // 2        # 3 head pairs (stacked 64+64 on partitions)
D1 = HD + 1            # v augmented with ones column
CC = C // P            # 6 output-projection chunks
EXP_SHIFT = -8.0       # exp(s*SCALE - 8): keeps fp16 exps in range
                       # (observed scaled-logit max ~15.2; ln(65504) ~ 11.09)

f32 = mybir.dt.float32
f16 = mybir.dt.float16
f32r = mybir.dt.float32r
EXP = mybir.ActivationFunctionType.Exp

_CACHE: dict = {}


def _build():
    nc = bacc.Bacc("TRN2", target_bir_lowering=False, debug=False, num_devices=8)

    xt = nc.dram_tensor("xt", [NBS, P, KC, NB], f16, kind="ExternalInput")  # x^T pre-blocked
    wq = nc.dram_tensor("wq", [PAIRS, P, KC, P], f16, kind="ExternalInput")
    wk = nc.dram_tensor("wk", [PAIRS, P, KC, P], f16, kind="ExternalInput")
    wv = nc.dram_tensor("wv", [P, KC, CL], f16, kind="ExternalInput")
    qb = nc.dram_tensor("qb", [PAIRS, P, N], f16, kind="ExternalInput")  # qbias^T + b_q
    kb = nc.dram_tensor("kb", [PAIRS, P, N], f16, kind="ExternalInput")
    vb = nc.dram_tensor("vb", [P, MC, CL], f16, kind="ExternalInput")     # vbias + b_v
    wp = nc.dram_tensor("wp", [P, PAIRS, C], f16, kind="ExternalInput")  # W_proj local rows
    ot = nc.dram_tensor("ot", [C, N], f16, kind="ExternalOutput")        # partial out^T

    ot_r = ot.ap().rearrange("(cc p) n -> p cc n", p=P)

    with tile.TileContext(nc) as tc:
        with (
            tc.tile_pool(name="persist", bufs=1) as pp,
            tc.tile_pool(name="stream", bufs=2) as sp,
            tc.tile_pool(name="ps", bufs=2, space="PSUM") as ps,
        ):
            # ---- persistent tiles ----
            wq_sb = pp.tile([P, PAIRS, KC, P], f16)
            wk_sb = pp.tile([P, PAIRS, KC, P], f16)
            wv_sb = pp.tile([P, KC, CL], f16)
            wp_sb = pp.tile([P, PAIRS, C], f16)
            qT = pp.tile([P, PAIRS, N], f16)    # q^T (pair-stacked heads)
            kT = pp.tile([P, PAIRS, N], f16)    # k^T
            v_aug = pp.tile([P, MC, HL, D1], f16)  # [v | 1] per m-chunk/head
            qb_sb = pp.tile([P, PAIRS, N], f16)    # qbias^T + b_q
            kb_sb = pp.tile([P, PAIRS, N], f16)
            vb_sb = pp.tile([P, MC, CL], f16)      # vbias + b_v per m-chunk
            ones_f32 = pp.tile([P, 1], f32)
            shift_f32 = pp.tile([P, 1], f32)    # exp bias (EXP_SHIFT)

            # DMA priority order. Every dma_start trigger costs ~650ns on its
            # issuing engine's queue, so inputs are loaded with FEW, BIG
            # transfers: the head-critical slices first on the sync queue,
            # the (large, persistent) bias tiles on the gpsimd queue.

            # PE warmup: enough dummy matmuls to flip the HAM clock gate to
            # 8/8 (2.4 GHz) while the first DMAs land, few enough that the
            # head's own matmuls aren't queued behind them.
            warm_a = pp.tile([P, P], f32r)
            warm_b = pp.tile([P, NB], f32r)
            nc.vector.memset(warm_a.bitcast(f32)[:], 0.0)
            nc.vector.memset(warm_b.bitcast(f32)[:], 0.0)
            wps = ps.tile([P, 2, NB], f32, tag="sps", name="warm_ps")
            for _ in range(6):
                nc.tensor.matmul(wps[:, 0, :], warm_a[:], warm_b[:], start=True, stop=True)

            nc.vector.memset(ones_f32[:], 1.0)
            nc.vector.memset(shift_f32[:], EXP_SHIFT)
            with nc.allow_low_precision(reason="ones column is exact in fp16"):
                nc.vector.tensor_copy(
                    v_aug[:, :, :, HD], ones_f32.to_broadcast([P, MC, HL])
                )
            # Pull the ~2.7us exp table load off the critical path: a dummy
            # 1-element exp while the head DMAs stream.
            act_warm = pp.tile([P, 1], f32)
            nc.scalar.activation(act_warm[:], ones_f32[:], EXP)

            # x^T blocks: all four DMAd upfront (bufs=4) so deferred qkv
            # pieces never wait on x.
            xt_blks = []
            for nb in range(NBS):
                t = sp.tile([P, KC, NB], f16, tag="xt", bufs=4, name=f"xt_{nb}")
                xt_blks.append(t)
            nc.sync.dma_start(xt_blks[0][:], xt.ap()[0])
            nc.sync.dma_start(wk_sb[:, 0], wk.ap()[0])
            nc.sync.dma_start(wq_sb[:, 0], wq.ap()[0])
            nc.sync.dma_start(wv_sb[:], wv.ap())
            # bias loads: pair-0 q/k and the first v quarter lead
            nc.gpsimd.dma_start(kb_sb[:, 0, :], kb.ap()[0])
            nc.gpsimd.dma_start(qb_sb[:, 0, :], qb.ap()[0])
            for q2 in range(2):
                ms = slice(q2 * 8, (q2 + 1) * 8)
                nc.gpsimd.dma_start(vb_sb[:, ms, :], vb.ap()[:, ms, :])
                if q2 == 0:
                    nc.gpsimd.dma_start(kb_sb[:, 1, :], kb.ap()[1])
                    nc.gpsimd.dma_start(qb_sb[:, 1, :], qb.ap()[1])
            nc.gpsimd.dma_start(kb_sb[:, 2, :], kb.ap()[2])
            nc.gpsimd.dma_start(qb_sb[:, 2, :], qb.ap()[2])

            # ---- qkv prologue pieces (emitted just-in-time) ----
            def qk_group(w_sb, b_sb, dst, nb, c3):
                """q or k for one (n-block, head-pair): 6 matmuls + bias add."""
                ns = slice(nb * NB, (nb + 1) * NB)
                tag = "q" if dst is qT else "k"
                mm = ps.tile([P, NB], f32, tag="mps", name=f"{tag}_{nb}_{c3}")
                for co in range(KC):
                    yield lambda co=co: nc.tensor.matmul(
                        mm[:], w_sb[:, c3, co, :], xt_blks[nb][:, co, :],
                        start=(co == 0), stop=(co == KC - 1),
                    )
                def add():
                    with nc.allow_low_precision(reason="fp16 matmul operand"):
                        nc.vector.tensor_add(dst[:, c3, ns], mm[:],
                                             b_sb[:, c3, ns])
                yield add

            def v_chunk(mchunk):
                """v for one 128-key chunk (all 6 heads): 6 matmuls + add."""
                nb, ch = mchunk // (NB // P), mchunk % (NB // P)
                cs = slice(ch * P, (ch + 1) * P)
                mm = ps.tile([P, NB], f32, tag="mps", name=f"v_{mchunk}")
                for co in range(KC):
                    yield lambda co=co: nc.tensor.matmul(
                        mm[:, :CL], xt_blks[nb][:, co, cs], wv_sb[:, co, :],
                        start=(co == 0), stop=(co == KC - 1),
                    )
                def add():
                    with nc.allow_low_precision(reason="fp16 matmul operand"):
                        nc.vector.tensor_add(
                            v_aug[:, mchunk, :, 0:HD], mm[:, :CL],
                            vb_sb[:, mchunk, :]
                        )
                yield add

            def drain(gen):
                for piece in gen:
                    piece()

            # Deferred prologue work: (deadline_iter, generator). A deadline
            # of d means "fully emitted while processing iteration <= d"
            # (before iteration d+1 emits the S that consumes it).
            sched = []

            def sched_qk(w_sb, b_sb, dst, nb, c3, deadline):
                sched.append((deadline, qk_group(w_sb, b_sb, dst, nb, c3)))

            def sched_v(mchunk, deadline):
                sched.append((deadline, v_chunk(mchunk)))

            # ---- head: minimum work before attention step (0,0,0) ----
            drain(qk_group(wk_sb, kb_sb, kT, 0, 0))
            drain(qk_group(wq_sb, qb_sb, qT, 0, 0))
            drain(v_chunk(0))
            for nb in range(1, NBS):
                nc.sync.dma_start(xt_blks[nb][:], xt.ap()[nb])
            for c3 in range(1, PAIRS):
                nc.sync.dma_start(wk_sb[:, c3], wk.ap()[c3])
                nc.sync.dma_start(wq_sb[:, c3], wq.ap()[c3])
            nc.sync.dma_start(wp_sb[:], wp.ap())

            # deadlines sit a few steps before the S that consumes each
            # group so the group's DVE bias-add clears the queue before the
            # boundary's normalize/proj pile-up.
            for mc in range(1, MC):
                sched_v(mc, mc)                             # AV(c3=0,nb=0,mc)
            for nb in range(1, NBS):
                sched_qk(wk_sb, kb_sb, kT, nb, 0, 4 * nb - 1)
            for c3 in range(1, PAIRS):
                for nb in range(NBS):
                    sched_qk(wk_sb, kb_sb, kT, nb, c3,
                             64 * c3 + 4 * nb - (12 if nb == 0 else 4))
            for nb in range(1, NBS):
                sched_qk(wq_sb, qb_sb, qT, nb, 0, 16 * nb - 4)
            for c3 in range(1, PAIRS):
                for nb in range(NBS):
                    sched_qk(wq_sb, qb_sb, qT, nb, c3,
                             64 * c3 + 16 * nb - (8 if nb == 0 else 4))
            sched.sort(key=lambda x: x[0])

            # ---- attention stream ----
            # c3-major: each head-pair phase spans 64 steps, so deferred
            # prologue groups and proj passes spread over 4x more steps.
            steps = [(nb, c3, mc)
                     for c3 in range(PAIRS)
                     for nb in range(NBS)
                     for mc in range(MC)]
            o_blks = {}
            ot_accs = {}
            u_cur = {}
            sps_tiles = {}
            exp_tiles = {}
            s_insts = {}

            def emit_s(i):
                nb, c3, mc = steps[i]
                ns = slice(nb * NB, (nb + 1) * NB)
                ms = slice(mc * P, (mc + 1) * P)
                sps = ps.tile([P, 2, NB], f32, tag="sps", name=f"s_{nb}_{c3}_{mc}")
                sps_tiles[i] = sps
                insts = []
                for hp in range(2):
                    hb = slice(hp * HD, (hp + 1) * HD)
                    bi = nc.tensor.matmul(
                        sps[:, hp, :], kT[hb, c3, ms], qT[hb, c3, ns],
                        start=True, stop=True,
                    )
                    insts.append(bi.ins)
                s_insts[i] = insts

            def emit_exp(i):
                nb, c3, mc = steps[i]
                exps = sp.tile([P, 2, NB], f16, tag="exps", bufs=4,
                               name=f"e_{nb}_{c3}_{mc}")
                exp_tiles[i] = exps
                with nc.allow_low_precision(reason="fp16 exps"):
                    nc.scalar.activation(exps[:], sps_tiles.pop(i)[:], EXP,
                                         bias=shift_f32[:], scale=SCALE)

            def emit_av(i):
                nb, c3, mc = steps[i]
                if mc == 0:
                    u_cur[0] = ps.tile([D1, NB], f32, tag="ups", name=f"u_{nb}_{c3}_0")
                    u_cur[1] = ps.tile([D1, NB], f32, tag="ups", name=f"u_{nb}_{c3}_1")
                exps = exp_tiles.pop(i)
                for hp in range(2):
                    bi = nc.tensor.matmul(
                        u_cur[hp][:], v_aug[:, mc, c3 * 2 + hp, :],
                        exps[:, hp, :],
                        start=(mc == 0), stop=(mc == MC - 1),
                    )
                    # Pin PE order: the (independent) S matmuls of step i+1
                    # must precede AV(i) in the PE FIFO so they run during
                    # exp(i) instead of behind AV(i)'s semaphore wait.
                    if hp == 0 and i + 1 in s_insts:
                        add_dep_helper(bi.ins, s_insts[i + 1][-1], sync=False,
                                       reason="keep S(i+1) ahead of AV(i)")

            def emit_normalize(nb, c3):
                # Drain the U psum banks early (den rows copied to one SBUF
                # tile, o_blk casts drain the out rows), then one recip over
                # both heads' denominators and one wide partition_broadcast;
                # the muls read per-head 64-row windows (32-aligned bases).
                o_blk = o_blks[nb]
                dent = sp.tile([1, 2, NB], f32, tag="den", bufs=4,
                               name=f"d_{nb}_{c3}")
                for hp in range(2):
                    u = u_cur[hp]
                    hb = slice(hp * HD, (hp + 1) * HD)
                    nc.vector.tensor_copy(dent[:, hp, :], u[HD:D1, :])
                    with nc.allow_low_precision(reason="fp16 matmul operand"):
                        nc.vector.tensor_copy(o_blk[hb, c3, :], u[0:HD, :])
                rec = sp.tile([1, 2, NB], f32, tag="rec", bufs=4,
                              name=f"r_{nb}_{c3}")
                nc.vector.reciprocal_approx_fast(rec[:], dent[:])
                bcs = []
                for hp in range(2):
                    bc = sp.tile([P, NB], f32, tag="bc", bufs=3,
                                 name=f"bc_{nb}_{c3}_{hp}")
                    nc.gpsimd.partition_broadcast(bc[:], rec[:, hp, :])
                    bcs.append(bc)
                for hp in range(2):
                    hb = slice(hp * HD, (hp + 1) * HD)
                    with nc.allow_low_precision(reason="fp16 matmul operand"):
                        nc.vector.tensor_mul(
                            o_blk[hb, c3, :], o_blk[hb, c3, :], bcs[hp][hb, :]
                        )

            # Output projection as three c3-major passes accumulated in SBUF:
            # pass p computes wp[pair p]^T o_blk[:, p, :] for all 6 output
            # chunks and adds into ot_acc; pass 2 also DMAs the chunk out.
            # Pass p only needs o_blk pair p (ready after normalize(nb, p)),
            # so passes 0/1 hide inside the stream and only pass 2 of the
            # last n-block trails the final attention step.
            def proj_pass01(nb):
                """wp[p0]^T o0 + wp[p1]^T o1 accumulated in PSUM, one drain."""
                o_blk = o_blks[nb]
                acc = ot_accs[nb]
                for cc in range(CC):
                    cs = slice(cc * P, (cc + 1) * P)
                    mm = ps.tile([P, NB], f32, tag="mps", name=f"p01_{nb}_{cc}")
                    for p in range(2):
                        yield lambda mm=mm, cs=cs, p=p: nc.tensor.matmul(
                            mm[:], wp_sb[:, p, cs], o_blk[:, p, :],
                            start=(p == 0), stop=(p == 1),
                        )
                    def red0(mm=mm, cc=cc):
                        with nc.allow_low_precision(reason="fp16 out partial"):
                            nc.vector.tensor_copy(acc[:, cc, :], mm[:])
                    yield red0

            def proj_pass2(nb):
                """last pair's contribution + output DMA per chunk."""
                ns = slice(nb * NB, (nb + 1) * NB)
                o_blk = o_blks[nb]
                acc = ot_accs[nb]
                for cc in range(CC):
                    cs = slice(cc * P, (cc + 1) * P)
                    mm = ps.tile([P, NB], f32, tag="mps", name=f"p2_{nb}_{cc}")
                    yield lambda mm=mm, cs=cs: nc.tensor.matmul(
                        mm[:], wp_sb[:, PAIRS - 1, cs], o_blk[:, PAIRS - 1, :],
                        start=True, stop=True,
                    )
                    def red(mm=mm, cc=cc):
                        with nc.allow_low_precision(reason="fp16 out partial"):
                            nc.vector.tensor_add(acc[:, cc, :], acc[:, cc, :],
                                                 mm[:])
                        nc.sync.dma_start(ot_r[:, cc, ns], acc[:, cc, :])
                    yield red
                o_blks.pop(nb)
                ot_accs.pop(nb)

            # opportunistic queue (earliest_iter, generator); proj passes
            # are appended as their o_blk pairs become final.
            oppo = []

            for i, (nb, c3, mc) in enumerate(steps):
                if mc == 0 and c3 == 0:
                    o_blks[nb] = sp.tile([P, PAIRS, NB], f16, tag="oblk",
                                         bufs=4, name=f"o_{nb}")
                    ot_accs[nb] = sp.tile([P, CC, NB], f16, tag="otacc",
                                          bufs=4, name=f"oa_{nb}")
                if i == 0:
                    emit_s(0)
                emit_exp(i)
                if i + 1 < len(steps):
                    emit_s(i + 1)
                emit_av(i)
                if mc == MC - 1:
                    emit_normalize(nb, c3)
                    # start passes well after the normalize chain has
                    # drained so their first matmul (which waits on the
                    # o_blk mul) never head-of-line blocks the PE FIFO
                    if c3 == 1:
                        oppo.append((i + 6, proj_pass01(nb)))
                    elif c3 == 2:
                        oppo.append((i + 6, proj_pass2(nb)))
                # deferred prologue groups due for the next step's S/AV
                while sched and sched[0][0] <= i + 1:
                    drain(sched.pop(0)[1])
                # opportunistic: a few proj-pass pieces per step
                budget = 3
                while budget and oppo:
                    if oppo[0][0] > i:
                        break
                    piece = next(oppo[0][1], None)
                    if piece is None:
                        oppo.pop(0)
                        continue
                    piece()
                    budget -= 1
            # drain everything left (only the last n-block's proj passes)
            while sched:
                drain(sched.pop(0)[1])
            for _, gen in oppo:
                drain(gen)

    nc.compile()
    return nc


def _get_nc():
    if "nc" not in _CACHE:
        _CACHE["nc"] = _build()
    return _CACHE["nc"]


def _prep_in_maps(x, qbias, kbias, vbias, W_qkv, b_qkv, W_proj):
    x = np.asarray(x, dtype=np.float32)
    qbias = np.asarray(qbias, dtype=np.float32)
    kbias = np.asarray(kbias, dtype=np.float32)
    vbias = np.asarray(vbias, dtype=np.float32)
    W_qkv = np.asarray(W_qkv, dtype=np.float32)
    b_qkv = np.asarray(b_qkv, dtype=np.float32)
    W_proj = np.asarray(W_proj, dtype=np.float32)

    f16c = lambda a: np.ascontiguousarray(a, dtype=np.float16)
    # x^T pre-blocked [NBS, P, KC, NB]: per-partition-contiguous DMA runs
    xts = [f16c(x[b].T.reshape(KC, P, NBS, NB).transpose(2, 1, 0, 3))
           for b in range(B)]

    def wblk(w):                       # [C, 384] -> [PAIRS, P, KC, P]
        return f16c(w.reshape(KC, P, PAIRS, P).transpose(2, 1, 0, 3))

    def rowblk(a, n_outer):            # [(n p), j] -> [P, n, j]
        return f16c(a.reshape(n_outer, P, a.shape[1]).transpose(1, 0, 2))
    in_maps = []
    for core in range(8):
        b, hg = core // 2, core % 2
        heads = slice(hg * HL, (hg + 1) * HL)
        qcols = slice(hg * CL, (hg + 1) * CL)
        kcols = slice(C + hg * CL, C + (hg + 1) * CL)
        vcols = slice(2 * C + hg * CL, 2 * C + (hg + 1) * CL)

        # per-head bias + projection bias, transposed to [pair, 128, N]
        qb_ = qbias[b, heads] + b_qkv[qcols].reshape(HL, 1, HD)   # [6, N, 64]
        kb_ = kbias[b, heads] + b_qkv[kcols].reshape(HL, 1, HD)
        qb_t = f16c(qb_.transpose(0, 2, 1)).reshape(PAIRS, P, N)
        kb_t = f16c(kb_.transpose(0, 2, 1)).reshape(PAIRS, P, N)
        # v bias in natural [N, 384] (heads side by side, matching Wv columns)
        vb_ = vbias[b, heads] + b_qkv[vcols].reshape(HL, 1, HD)   # [6, N, 64]
        vb_n = f16c(vb_.transpose(1, 0, 2)).reshape(N, CL)

        in_maps.append({
            "xt": xts[b],
            "wq": wblk(W_qkv[:, qcols]),
            "wk": wblk(W_qkv[:, kcols]),
            "wv": rowblk(W_qkv[:, vcols], KC),
            "qb": qb_t,
            "kb": kb_t,
            "vb": rowblk(vb_n, MC),
            "wp": rowblk(W_proj[hg * CL:(hg + 1) * CL, :], PAIRS),
        })
    return in_maps


def kernel(x, qbias, kbias, vbias, W_qkv, b_qkv, W_proj, b_proj, **run_kwargs):
    nc = _get_nc()
    in_maps = _prep_in_maps(x, qbias, kbias, vbias, W_qkv, b_qkv, W_proj)
    res = run_bass_kernel_spmd(nc, in_maps, core_ids=list(range(8)), **run_kwargs)
    _CACHE["last_results"] = res

    b_proj = np.asarray(b_proj, dtype=np.float32)
    out = np.empty((B, N, C), dtype=np.float32)
    for b in range(B):
        part = (res.results[2 * b]["ot"].astype(np.float32)
                + res.results[2 * b + 1]["ot"].astype(np.float32))  # [C, N]
        out[b] = part.T + b_proj
    return out
